# revision 1
# baseline (speedup 1.0000x reference)
"""CostVolume Trainium2 kernel (Bass/Tile), SPMD over 8 NeuronCores.

Problem (hardcoded shapes):
  x, y: [2, 64, 96, 320] f32.  GROUP=8, cpg=8, MAXDISP=48, D=49.
  out:  [2, 8, 49, 96, 320] f32
  out[b,g,k,h,j] = sum_c | xn[b,g,c,h,j] - yn_pad[b,g,c,h,j+48-k] |
  where xn/yn are channel-group L2-normalized (norm over cpg=8), and yn_pad
  has 48 zero columns prepended along w.

Sharding: the 16 (b, g) pairs are fully independent -> 2 per core.

Per-core layout:
  SBUF tile per bg: partition p = c*16 + hh (c in 0..8, hh in 0..16),
  free = (ht in 0..6, w), with h = ht*16 + hh.
  - normalize:  ssum[hh, n] = sum_c x^2  via PE matmul with a 0/1 selection
    matrix (f32r for full rate), rsqrt on ACT, broadcast back to 128
    partitions via a second PE matmul, multiply on DVE (cast to fp16).
  - main loop over (bg, k): DVE fp16 subtract (2x mode), abs alternating
    between ACT (Abs activation) and DVE (bitwise-and 0x7fff on uint16 view),
    channel-reduce via PE matmuls into PSUM [96, 320], ACT copy PSUM->SBUF,
    DMA out.
  Odd k would mis-align the fp16 2x DVE mode (offset 48-k), so two copies of
  the padded yn are kept: one with 48 leading zeros (even k), one with 49
  (odd k).
"""

import numpy as np

B, C, H, W = 2, 64, 96, 320
GROUP = 8
CPG = C // GROUP          # 8
MAXDISP = 48
D = MAXDISP + 1           # 49
NCORES = 8
BG_TOTAL = B * GROUP      # 16
BG_PER_CORE = BG_TOTAL // NCORES  # 2
HH = 16                   # partition sub-index of h
HT = H // HH              # 6
PW = 376                  # padded y width (48 zeros + 320 + 8 slack)

_PROG = None  # cached (nc, names)


def _constants():
    # s0[p, m] = 1 iff p % 16 == m   (channel-group reduce, K=128 -> M=16)
    s0 = np.zeros((128, HH), np.float32)
    for p in range(128):
        s0[p, p % HH] = 1.0
    # r0[q, p] = 1 iff p % 16 == q   (broadcast [16,n] -> [128,n], K=16)
    r0 = np.zeros((HH, 128), np.float32)
    for p in range(128):
        r0[p % HH, p] = 1.0
    # s6[p, t, m] = 1 iff m == 16*t + p % 16 — per-ht reduce weights writing
    # the full [96, N] PSUM tile (PE base partitions must be 0/32/64, so the
    # six ht blocks accumulate into one tile instead of partition-offset
    # slices).
    s6 = np.zeros((128, HT, H), np.float16)
    for p in range(128):
        for t in range(HT):
            s6[p, t, HH * t + p % HH] = 1.0
    return {
        "s6f16": s6,
        "s0f32": s0,
        "r0f32": r0.astype(np.float32),
    }


def _build():
    global _PROG
    if _PROG is not None:
        return _PROG

    import concourse.bacc as bacc
    import concourse.bass as bass
    import concourse.mybir as mybir
    import concourse.tile as tile

    f32 = mybir.dt.float32
    f32r = mybir.dt.float32r
    f16 = mybir.dt.float16
    u16 = mybir.dt.uint16
    AF = mybir.ActivationFunctionType
    ALU = mybir.AluOpType

    nc = bacc.Bacc("TRN2", target_bir_lowering=False, debug=False)

    # inputs are host-pre-transposed to [bg, (c hh)=128, ht, w] so each
    # per-(bg, tensor) load is ONE contiguous DMA (the DMA AP balancer
    # allows at most 3 dims per side).
    xin = nc.dram_tensor("x", [BG_PER_CORE, 128, HT, W], f32, kind="ExternalInput")
    yin = nc.dram_tensor("y", [BG_PER_CORE, 128, HT, W], f32, kind="ExternalInput")
    s6f16_d = nc.dram_tensor("s6f16", [128, HT, H], f16, kind="ExternalInput")
    # f32r (tf32-like) so the normalization matmuls run at full PE rate; the
    # BIR verifier requires f32r matmul operands to be produced as f32r.
    s0f32_d = nc.dram_tensor("s0f32", [128, HH], f32r, kind="ExternalInput")
    r0f32_d = nc.dram_tensor("r0f32", [HH, 128], f32r, kind="ExternalInput")
    out_d = nc.dram_tensor("out", [BG_PER_CORE, D, H, W], f32, kind="ExternalOutput")

    x_v = xin.ap()
    y_v = yin.ap()
    out_v = out_d.ap()

    with tile.TileContext(nc) as tc:
        with (
            tc.tile_pool(name="const", bufs=1) as constp,
            tc.tile_pool(name="stage", bufs=4) as stagep,
            tc.tile_pool(name="norm", bufs=2) as normp,
            tc.tile_pool(name="keep", bufs=1) as keepp,
            tc.tile_pool(name="nps", bufs=2, space="PSUM") as npsp,
            tc.tile_pool(name="mmps", bufs=2, space="PSUM") as mmpsp,
            tc.tile_pool(name="dpool", bufs=4) as dpool,
            tc.tile_pool(name="apool", bufs=4) as apool,
            tc.tile_pool(name="opool", bufs=8) as opool,
        ):
            s6_16 = constp.tile([128, HT, H], f16, tag="s6f16")
            s0_32 = constp.tile([128, HH], f32r, tag="s0f32")
            r0_32 = constp.tile([HH, 128], f32r, tag="r0f32")
            nc.sync.dma_start(s6_16[:], s6f16_d.ap())
            nc.sync.dma_start(s0_32[:], s0f32_d.ap())
            nc.sync.dma_start(r0_32[:], r0f32_d.ap())

            # persistent normalized tiles
            xn = []    # per bg: [128, HT, W] fp16
            xa = []    # per bg: [128, HT, W] fp16, |xn| (pad-region reduce src)
            ynp0 = []  # per bg: [128, HT, PW] fp16, 48 leading zeros
            ynp1 = []  # per bg: [128, HT, PW] fp16, 49 leading zeros
            for bg in range(BG_PER_CORE):
                xn.append(keepp.tile([128, HT, W], f16, tag=f"xn{bg}", name=f"xn{bg}"))
                xa.append(keepp.tile([128, HT, W], f16, tag=f"xa{bg}", name=f"xa{bg}"))
                ynp0.append(keepp.tile([128, HT, PW], f16, tag=f"ynp0{bg}", name=f"ynp0{bg}"))
                ynp1.append(keepp.tile([128, HT, PW], f16, tag=f"ynp1{bg}", name=f"ynp1{bg}"))

            # ---------------- normalization phase ----------------
            for bg in range(BG_PER_CORE):
                # only the left-pad columns are ever read as zeros; the
                # tail columns past MAXDISP+1+W are never read at all.
                nc.vector.memset(ynp0[bg][:, :, 0:MAXDISP], 0.0)
                nc.vector.memset(ynp1[bg][:, :, 0:MAXDISP + 1], 0.0)
                for is_y in (0, 1):
                    src_v = y_v if is_y else x_v
                    # load + square in ht-pair chunks so normalization of
                    # the first rows starts while the rest still streams in
                    raw = stagep.tile([128, HT, W], f32, tag="raw")
                    sq = stagep.tile([128, HT, W], f32r, tag="sq")
                    for t2 in range(HT // 2):
                        sl = slice(2 * t2, 2 * t2 + 2)
                        nc.sync.dma_start(raw[:, sl], src_v[bg][:, sl])
                        nc.scalar.activation(
                            sq[:, sl].rearrange("p a b -> p (a b)"),
                            raw[:, sl].rearrange("p a b -> p (a b)"),
                            AF.Square,
                        )
                    rs = normp.tile([HH, HT, W], f32r, tag="rs")
                    for t in range(HT):
                        ssum = npsp.tile([HH, W], f32, tag="ssum")
                        nc.tensor.matmul(
                            ssum[:],
                            s0_32[:],
                            sq[:, t, :],
                            start=True,
                            stop=True,
                        )
                        # 1/sqrt(|ssum|); ssum >= 0. (Rsqrt/Reciprocal are
                        # gated in bass; this variant lives in the same ACT
                        # table set as Abs/Square/Copy so no table reloads.)
                        nc.scalar.activation(rs[:, t, :], ssum[:], AF.Abs_reciprocal_sqrt)
                    for t in range(HT):
                        rb = npsp.tile([128, W], f32, tag="rb")
                        nc.tensor.matmul(
                            rb[:],
                            r0_32[:],
                            rs[:, t, :],
                            start=True,
                            stop=True,
                        )
                        if is_y:
                            nc.vector.tensor_mul(
                                ynp0[bg][:, t, MAXDISP:MAXDISP + W], raw[:, t, :], rb[:]
                            )
                        else:
                            nc.vector.tensor_mul(xn[bg][:, t, :], raw[:, t, :], rb[:])
                    if not is_y:
                        nc.vector.tensor_scalar(
                            xa[bg].rearrange("p a b -> p (a b)").bitcast(u16),
                            xn[bg].rearrange("p a b -> p (a b)").bitcast(u16),
                            0x7FFF,
                            None,
                            op0=ALU.bitwise_and,
                        )
                    if is_y:
                        # odd-k copy: same values shifted one element right; a
                        # byte-addressed SBUF->SBUF DMA costs no engine time.
                        nc.sync.dma_start(
                            ynp1[bg][:, :, MAXDISP + 1:MAXDISP + 1 + W],
                            ynp0[bg][:, :, MAXDISP:MAXDISP + W],
                        )

            # ---------------- main loop ----------------
            # k processed in pairs sharing one 2-bank PSUM tile so the
            # PSUM->SBUF copy and the store DMA amortize over 2 outputs.
            # abs runs on DVE (bitwise-and 0x7fff, 4x mode) for ~38% of
            # iterations and on ACT (Abs) for the rest — balances the two
            # engines (DVE also carries the subtracts, ACT the copies).
            # per-k: DVE fp16 subtract (2x), abs on DVE (bitwise-and 0x7fff
            # on a uint16 view, 4x mode) for 49 of 98 iterations and on ACT
            # (Abs) for the rest — balances the engines (DVE also carries the
            # subtracts; ACT the PSUM->SBUF copies). k-pairs share one 2-bank
            # PSUM tile so copy + store amortize over two outputs.
            it = 0
            for bg in range(BG_PER_CORE):
                for k0 in range(0, D, 2):
                    ks = [k0] if k0 + 1 >= D else [k0, k0 + 1]
                    nj = len(ks)
                    ps = mmpsp.tile([H, 2, 512], f32, tag="ps")
                    for j, k in enumerate(ks):
                        if k % 2 == 0:
                            # valid window only; all offsets stay 4B-aligned
                            wv = W - k
                            d = dpool.tile([128, HT, W], f16, tag="d", name="d")
                            nc.vector.tensor_sub(
                                d[:, :, 0:wv],
                                xn[bg][:, :, k:W],
                                ynp0[bg][:, :, MAXDISP:MAXDISP + wv],
                            )
                        else:
                            wv = W
                            d = dpool.tile([128, HT, W], f16, tag="d", name="d")
                            nc.vector.tensor_sub(
                                d[:], xn[bg][:], ynp1[bg][:, :, MAXDISP + 1 - k:MAXDISP + 1 - k + W]
                            )
                        absd = apool.tile([128, HT, W], f16, tag="absd", name="absd")
                        dv = d[:, :, 0:wv]
                        av = absd[:, :, 0:wv]
                        if (it * 49) // 98 != ((it + 1) * 49) // 98:
                            nc.vector.tensor_scalar(
                                av.bitcast(u16), dv.bitcast(u16), 0x7FFF, None,
                                op0=ALU.bitwise_and,
                            )
                        else:
                            nc.scalar.activation(av, dv, AF.Abs)
                        for t in range(HT):
                            if k % 2 == 0 and k > 0:
                                nc.tensor.matmul(
                                    ps[:, j, 0:k],
                                    s6_16[:, t, :],
                                    xa[bg][:, t, 0:k],
                                    start=(t == 0),
                                    stop=False,
                                )
                                nc.tensor.matmul(
                                    ps[:, j, k:W],
                                    s6_16[:, t, :],
                                    absd[:, t, 0:wv],
                                    start=False,
                                    stop=(t == HT - 1),
                                )
                            else:
                                nc.tensor.matmul(
                                    ps[:, j, 0:wv],
                                    s6_16[:, t, :],
                                    absd[:, t, 0:wv],
                                    start=(t == 0),
                                    stop=(t == HT - 1),
                                )
                        it += 1
                    ob = opool.tile([H, 2, W], f32, tag="ob")
                    nc.scalar.activation(
                        ob[:, 0:nj, :], ps[:, 0:nj, 0:W], AF.Copy
                    )
                    nc.sync.dma_start(
                        out_v[bg, k0:k0 + nj].rearrange("k h w -> h k w"), ob[:, 0:nj, :]
                    )

    nc.compile()
    _PROG = nc
    return nc


def run(x, y, trace=False, trace_kwargs=None):
    """x, y: full [2, 64, 96, 320] f32. Returns (out [2,8,49,96,320] f32, results)."""
    from concourse import bass_utils

    nc = _build()
    consts = _constants()
    def _prep(a):
        # [16, c=8, (ht hh)=96, w] -> [16, (c hh)=128, ht=6, w]
        a = np.asarray(a, np.float32).reshape(BG_TOTAL, CPG, HT, HH, W)
        return np.ascontiguousarray(a.transpose(0, 1, 3, 2, 4)).reshape(
            BG_TOTAL, 128, HT, W
        )

    xr = _prep(x)
    yr = _prep(y)
    in_maps = []
    for i in range(NCORES):
        sl = slice(i * BG_PER_CORE, (i + 1) * BG_PER_CORE)
        in_maps.append(
            {
                "x": np.ascontiguousarray(xr[sl]),
                "y": np.ascontiguousarray(yr[sl]),
                **consts,
            }
        )
    res = bass_utils.run_bass_kernel_spmd(
        nc,
        in_maps,
        core_ids=list(range(NCORES)),
        trace=trace,
        **(trace_kwargs or {}),
    )
    full = np.concatenate([r["out"] for r in res.results], axis=0)
    out = full.reshape(B, GROUP, D, H, W).astype(np.float32)
    return out, res


def kernel(x, y):
    out, _ = run(x, y, trace=False)
    return out

